# revision 7
# baseline (speedup 1.0000x reference)
"""NCNPredictor v7c: packed-fp8 binary masks, transposed dma_gather, PE sums.

Per edge (i,j), per core's 1250-column slice, y_k = x @ Wxs[(k+1)D:(k+2)D]:
  out = S p y0 + S q y1 + S r y2 - S p q y0 - S p r y2 + xij + b
  p = A01[i]A01[j], q = A1[i]A1[j], r = A012[i]A012[j]

B = [A01 | A1 | A012] stored as fp8e4 {0, 1.0} bytes, blocks zero-padded
1250->1280, PACKED PAIRWISE into a bf16-typed [N, 1920] tensor (each bf16
value holds two adjacent fp8 mask bytes; the gather and the bitwise ops
never interpret values, and the PE reads the masks back through an fp8
bitcast). Per 512-edge batch, two dma_gather(transpose=True) calls fetch
the i-rows and j-rows n-on-partitions: gt[p, c, e] is the u16 pair for
nodes n = 256c + 2p + {0,1} of edge e's row. Mask products are exact
bitwise ANDs (DVE 2x); the PE contracts over n with per-(chunk, parity)
y-weight columns (stride-2 fp8 rhs, bf16 lhsT, signs folded into the
weights), accumulating all five terms into one PSUM row per batch; ACT
copies PSUM->SBUF. Bias added host-side.
"""

import sys
from contextlib import ExitStack

import numpy as np

sys.path.insert(0, "/opt/trn_rl_repo")

import concourse.bass as bass
import concourse.tile as tile
from concourse import bacc, library_config, mybir
from concourse.bass_utils import run_bass_kernel_spmd

N = 10000
D = 128
E = 8192
NCORES = 8
NCOL = N // NCORES          # 1250 data cols per block
BPAD = 1280                 # padded block width (fp8 cols)
ELP = 3 * BPAD              # 3840 fp8 cols per row
ELP2 = ELP // 2             # 1920 packed bf16 units per row
C15 = ELP // 256            # 15 u16 chunks (128 units each)
CB5 = BPAD // 256           # 5 u16 chunks per block
P = 128
E2 = 512                    # edges per batch
NB = E // E2                # 16 batches
NIW = E2 // 16              # idx cols per gather list (16-wrap)
E_OWN = E // NCORES
ET_OWN = E_OWN // P
F32 = mybir.dt.float32
BF16 = mybir.dt.bfloat16
FP8 = mybir.dt.float8e4
U16 = mybir.dt.uint16
I16 = mybir.dt.int16
I32 = mybir.dt.int32
MUL = mybir.AluOpType.mult
AND = mybir.AluOpType.bitwise_and

_CACHE = {}


def _build_nc():
    nc = bacc.Bacc(num_swdge_queues=4)

    bcat = nc.declare_dram_parameter("bcat", [N, ELP2], BF16, False)
    idxs = nc.declare_dram_parameter("idxs", [P, NB * 2 * NIW], I16, False)
    # fp8 hi/lo weight pairs packed two-per-bf16: bf16 col c = fp8 (s0, s1)
    # parity weights of u16-chunk c.
    yw134h = nc.declare_dram_parameter("yw134h", [P, C15], BF16, False)
    yw134l = nc.declare_dram_parameter("yw134l", [P, C15], BF16, False)
    yw2h = nc.declare_dram_parameter("yw2h", [P, CB5], BF16, False)
    yw2l = nc.declare_dram_parameter("yw2l", [P, CB5], BF16, False)
    yw5h = nc.declare_dram_parameter("yw5h", [P, CB5], BF16, False)
    yw5l = nc.declare_dram_parameter("yw5l", [P, CB5], BF16, False)
    xw = nc.declare_dram_parameter("xw", [N, D], F32, False)
    xr = nc.declare_dram_parameter("xr", [N, D], F32, False)
    iow = nc.declare_dram_parameter("iow", [P, ET_OWN], I32, False)
    jow = nc.declare_dram_parameter("jow", [P, ET_OWN], I32, False)

    out_cn = nc.declare_dram_parameter("out_cn", [1, E], F32, True)
    out_xij = nc.declare_dram_parameter("out_xij", [P, ET_OWN], F32, True)

    with tile.TileContext(nc) as tc, ExitStack() as ctx:
        nc.gpsimd.load_library(library_config.mlp)

        cst = ctx.enter_context(tc.tile_pool(name="cst", bufs=1))
        idxs_t = cst.tile([P, NB * 2 * NIW], I16, name="idxs_t")
        nc.sync.dma_start(idxs_t[:], idxs[:])
        w134_t = cst.tile([P, 2 * C15], BF16, name="w134_t")
        nc.sync.dma_start(w134_t[:], yw134[:])
        w2_t = cst.tile([P, 2 * CB5], BF16, name="w2_t")
        nc.sync.dma_start(w2_t[:], yw2[:])
        w5_t = cst.tile([P, 2 * CB5], BF16, name="w5_t")
        nc.sync.dma_start(w5_t[:], yw5[:])
        iow_t = cst.tile([P, ET_OWN], I32, name="iow_t")
        nc.sync.dma_start(iow_t[:], iow[:])
        jow_t = cst.tile([P, ET_OWN], I32, name="jow_t")
        nc.sync.dma_start(jow_t[:], jow[:])

        acc = ctx.enter_context(tc.tile_pool(name="acc", bufs=1))
        out_acc = acc.tile([1, E], F32, name="out_acc")
        oxa_acc = acc.tile([P, ET_OWN], F32, name="oxa_acc")

        gat = ctx.enter_context(tc.tile_pool(name="gat", bufs=2))
        up = ctx.enter_context(tc.tile_pool(name="up", bufs=2))
        scr = ctx.enter_context(tc.tile_pool(name="scr", bufs=2))
        psp = ctx.enter_context(
            tc.tile_pool(name="psp", bufs=2, space=bass.MemorySpace.PSUM)
        )

        for b in range(NB):
            ic = (2 * b) * NIW
            jc = (2 * b + 1) * NIW
            gi = gat.tile([P, C15, E2], BF16, name="gi")
            nc.gpsimd.dma_gather(
                gi[:], bcat[:], idxs_t[:, ic : ic + NIW], E2, E2, ELP2,
                transpose=True,
            )
            gj = gat.tile([P, C15, E2], BF16, name="gj")
            nc.gpsimd.dma_gather(
                gj[:], bcat[:], idxs_t[:, jc : jc + NIW], E2, E2, ELP2,
                transpose=True,
            )
            ut = up.tile([P, C15, E2], BF16, name="ut")
            nc.vector.tensor_tensor(
                out=ut[:].bitcast(U16), in0=gi[:].bitcast(U16),
                in1=gj[:].bitcast(U16), op=AND,
            )
            vt = scr.tile([P, CB5, E2], BF16, name="vt")
            nc.vector.tensor_tensor(
                out=vt[:].bitcast(U16), in0=ut[:, 0:CB5, :].bitcast(U16),
                in1=ut[:, CB5 : 2 * CB5, :].bitcast(U16), op=AND,
            )
            wt = scr.tile([P, CB5, E2], BF16, name="wt")
            nc.vector.tensor_tensor(
                out=wt[:].bitcast(U16), in0=ut[:, 2 * CB5 : C15, :].bitcast(U16),
                in1=ut[:, 0:CB5, :].bitcast(U16), op=AND,
            )

            utf = ut[:].bitcast(FP8)   # [P, C15, 2*E2]
            vtf = vt[:].bitcast(FP8)
            wtf = wt[:].bitcast(FP8)
            ps = psp.tile([1, E2], F32, name="ps")
            k = 0
            for c in range(C15):
                for s in range(2):
                    nc.tensor.matmul(
                        ps[:], w134_t[:, 2 * c + s : 2 * c + s + 1],
                        utf[:, c, s : 2 * E2 : 2],
                        start=(k == 0), stop=False,
                    )
                    k += 1
            for c in range(CB5):
                for s in range(2):
                    nc.tensor.matmul(
                        ps[:], w2_t[:, 2 * c + s : 2 * c + s + 1],
                        vtf[:, c, s : 2 * E2 : 2],
                        start=False, stop=False,
                    )
            for c in range(CB5):
                for s in range(2):
                    last = c == CB5 - 1 and s == 1
                    nc.tensor.matmul(
                        ps[:], w5_t[:, 2 * c + s : 2 * c + s + 1],
                        wtf[:, c, s : 2 * E2 : 2],
                        start=False, stop=last,
                    )
            nc.scalar.copy(out_acc[:, b * E2 : (b + 1) * E2], ps[:])

        for eo in range(ET_OWN):
            xi_t = gat.tile([P, D], F32, name="xi_t")
            nc.gpsimd.indirect_dma_start(
                out=xi_t[:], out_offset=None, in_=xw[:],
                in_offset=bass.IndirectOffsetOnAxis(ap=iow_t[:, eo : eo + 1], axis=0),
            )
            xj_t = gat.tile([P, D], F32, name="xj_t")
            nc.gpsimd.indirect_dma_start(
                out=xj_t[:], out_offset=None, in_=xr[:],
                in_offset=bass.IndirectOffsetOnAxis(ap=jow_t[:, eo : eo + 1], axis=0),
            )
            oxe = scr.tile([P, D], F32, name="oxe")
            nc.vector.scalar_tensor_tensor(
                out=oxe[:], in0=xi_t[:], scalar=1.0, in1=xj_t[:],
                op0=MUL, op1=MUL, accum_out=oxa_acc[:, eo : eo + 1],
            )

        nc.sync.dma_start(out_cn[:], out_acc[:])
        nc.sync.dma_start(out_xij[:], oxa_acc[:])

    return nc


def get_nc():
    if "nc" not in _CACHE:
        nc = _build_nc()
        nc.compile()
        _CACHE["nc"] = nc
    return _CACHE["nc"]


def _wrap16(lst):
    """[P, len//16] int16: idx k at [k % 16 (replicated mod 16), k // 16]."""
    return np.tile(lst.reshape(-1, 16).T, (P // 16, 1))


def make_in_maps(x, adj_0_1, adj_1, adj_0_1_2, tar_ei, Wxs, bxs):
    import ml_dtypes

    bf = ml_dtypes.bfloat16
    f8 = mybir.dt.np(FP8)
    x32 = np.ascontiguousarray(x, dtype=np.float32)
    wxs = np.asarray(Wxs, dtype=np.float32)
    w0 = wxs[0:D, 0]
    wy = np.concatenate(
        [wxs[D : 2 * D], wxs[2 * D : 3 * D], wxs[3 * D : 4 * D]], axis=1
    )
    y = x32 @ wy  # [N, 3]
    xwf = np.ascontiguousarray(x32 * w0[None, :])

    ii16 = tar_ei[0].astype(np.int16)
    jj16 = tar_ei[1].astype(np.int16)
    idxs = np.zeros((P, NB * 2 * NIW), np.int16)
    for b in range(NB):
        es = slice(b * E2, (b + 1) * E2)
        idxs[:, (2 * b) * NIW : (2 * b + 1) * NIW] = _wrap16(ii16[es])
        idxs[:, (2 * b + 1) * NIW : (2 * b + 2) * NIW] = _wrap16(jj16[es])

    a01 = np.asarray(adj_0_1, dtype=np.float32)
    a1 = np.asarray(adj_1, dtype=np.float32)
    a012 = np.asarray(adj_0_1_2, dtype=np.float32)

    ii32 = tar_ei[0].astype(np.int32)
    jj32 = tar_ei[1].astype(np.int32)

    in_maps = []
    for c in range(NCORES):
        c0 = c * NCOL
        cs = slice(c0, c0 + NCOL)
        bfp8 = np.zeros((N, ELP), dtype=f8)
        bfp8[:, 0:NCOL] = a01[:, cs]
        bfp8[:, BPAD : BPAD + NCOL] = a1[:, cs]
        bfp8[:, 2 * BPAD : 2 * BPAD + NCOL] = a012[:, cs]
        bcat = bfp8.view(bf)  # packed pairs, bit-identical bytes

        ypad = np.zeros((3, BPAD), np.float32)
        ypad[0, 0:NCOL] = y[cs, 0]
        ypad[1, 0:NCOL] = y[cs, 1]
        ypad[2, 0:NCOL] = y[cs, 2]
        ycat = np.concatenate([ypad[0], ypad[1], ypad[2]])  # [3840]

        # weight col 2c+s at partition p weights node g = 256c + 2p + s
        pidx = np.arange(P)
        w134 = np.zeros((P, 2 * C15), np.float32)
        for ch in range(C15):
            for s in range(2):
                w134[:, 2 * ch + s] = ycat[256 * ch + 2 * pidx + s]
        w2 = np.zeros((P, 2 * CB5), np.float32)
        w5 = np.zeros((P, 2 * CB5), np.float32)
        for ch in range(CB5):
            for s in range(2):
                w2[:, 2 * ch + s] = -ypad[0][256 * ch + 2 * pidx + s]
                w5[:, 2 * ch + s] = -ypad[2][256 * ch + 2 * pidx + s]

        esl = slice(c * E_OWN, (c + 1) * E_OWN)
        iow = np.ascontiguousarray(ii32[esl].reshape(ET_OWN, P).T)
        jow = np.ascontiguousarray(jj32[esl].reshape(ET_OWN, P).T)
        in_maps.append({
            "bcat": np.ascontiguousarray(bcat),
            "idxs": idxs,
            "yw134": np.ascontiguousarray(w134.astype(bf)),
            "yw2": np.ascontiguousarray(w2.astype(bf)),
            "yw5": np.ascontiguousarray(w5.astype(bf)),
            "xw": xwf,
            "xr": x32,
            "iow": iow,
            "jow": jow,
        })
    return in_maps


def combine_results(results, b):
    out = np.zeros((E,), dtype=np.float64)
    for c in range(NCORES):
        out += results[c]["out_cn"].astype(np.float64).reshape(E)
        esl = slice(c * E_OWN, (c + 1) * E_OWN)
        out[esl] += results[c]["out_xij"].astype(np.float64).T.reshape(E_OWN)
    return (out + b).astype(np.float32).reshape(E, 1)


def kernel(x, adj_0_1, adj_1, adj_0_1_2, tar_ei, Wxs, bxs):
    nc = get_nc()
    in_maps = make_in_maps(x, adj_0_1, adj_1, adj_0_1_2, tar_ei, Wxs, bxs)
    res = run_bass_kernel_spmd(nc, in_maps, list(range(NCORES)))
    b = float(np.asarray(bxs, dtype=np.float32).reshape(-1)[0])
    return combine_results(res.results, b)


# revision 22
# speedup vs baseline: 20.4219x; 20.4219x over previous
"""NCNPredictor v8: 3-bit-packed adjacency, transposed dma_gather, PE sums.

Per edge (i,j), per core's 1250-column slice, y_k = x @ Wxs[(k+1)D:(k+2)D]:
  out = S p y0 + S q y1 + S r y2 - S p q y0 - S p r y2 + xij + b
  p = A01[i]A01[j], q = A1[i]A1[j], r = A012[i]A012[j]

All three adjacency bits of node n live in ONE byte: v = 4*A01 + 2*A1 +
A012 (zero-padded 1250->1280 cols), exposed as a bf16-typed [N, 640]
tensor (the gather and bitwise ops never interpret values). Per 512-edge
batch, two dma_gather(transpose=True) calls fetch i-rows and j-rows
n-on-partitions: gt[p, c, e] is the u16 byte-pair for nodes
n = 256c + 2p + {0,1}. Then, all exact:
  t  = gi & gj                 (packed p,q,r product bits per byte)
  up = (t & 0x0404) << 4       (p-mask as fp8 bytes {0, 2.0})
  uq = (t & 0x0202) << 5       (q-mask)
  ur = (t & 0x0101) << 6       (r-mask)
  m1 = up & (uq ^ 0x4040)      (p & ~q mask — the reference's cn_0)
  m2 = ur & (up ^ 0x4040)      (r & ~p mask — cn_2)
The PE contracts over n per (chunk, parity) with bf16 weight columns
(w = y_k/2, mask value 2.0 folded in): three plain fp8-rhs matmul passes
(m1*y0, uq*y1, m2*y2) accumulate into one PSUM row per batch; ACT copies
PSUM->SBUF. Bias added host-side.
"""

import sys
from contextlib import ExitStack

import numpy as np

sys.path.insert(0, "/opt/trn_rl_repo")

import concourse.bass as bass
import concourse.tile as tile
from concourse import bacc, library_config, mybir
from concourse.bass_utils import run_bass_kernel_spmd

N = 10000
D = 128
E = 8192
NCORES = 8
NCOL = N // NCORES          # 1250 data cols (nodes) per core
BPAD = 1280                 # padded width (bytes per row)
BP2 = BPAD // 2             # 640 bf16 units per row
C5 = BPAD // 256            # 5 u16 chunks
P = 128
E2 = 512                    # edges per batch
NB = E // E2                # 16 batches
NIW = E2 // 16              # idx cols per gather list (16-wrap)
E_OWN = E // NCORES
ET_OWN = E_OWN // P
F32 = mybir.dt.float32
BF16 = mybir.dt.bfloat16
FP8 = mybir.dt.float8e4
U16 = mybir.dt.uint16
I16 = mybir.dt.int16
I32 = mybir.dt.int32
MUL = mybir.AluOpType.mult
AND = mybir.AluOpType.bitwise_and
SHL = mybir.AluOpType.logical_shift_left

_CACHE = {}

# (mask name, weight set) per PE pass, in PSUM-accumulation order
TERMS = ("m1", "uq", "m2")


def _build_nc(reps=1):
    nc = bacc.Bacc(num_swdge_queues=4)

    bcat = nc.declare_dram_parameter("bcat", [N, BP2], BF16, False)
    idxs = nc.declare_dram_parameter("idxs", [P, NB * 2 * NIW], I16, False)
    wd = {
        t: nc.declare_dram_parameter(f"w_{t}", [P, 2 * C5], BF16, False)
        for t in TERMS
    }
    xw = nc.declare_dram_parameter("xw", [N, D], F32, False)
    xr = nc.declare_dram_parameter("xr", [N, D], F32, False)
    iow = nc.declare_dram_parameter("iow", [P, ET_OWN], I32, False)
    jow = nc.declare_dram_parameter("jow", [P, ET_OWN], I32, False)

    out_cn = nc.declare_dram_parameter("out_cn", [1, E], F32, True)
    out_xij = nc.declare_dram_parameter("out_xij", [P, ET_OWN], F32, True)

    with tile.TileContext(nc) as tc, ExitStack() as ctx:
        nc.gpsimd.load_library(library_config.mlp)

        cst = ctx.enter_context(tc.tile_pool(name="cst", bufs=1))
        idxs_t = cst.tile([P, NB * 2 * NIW], I16, name="idxs_t")
        nc.sync.dma_start(idxs_t[:], idxs[:])
        wts = {}
        for nm, dram in wd.items():
            t = cst.tile([P, 2 * C5], BF16, name=f"wt_{nm}")
            nc.sync.dma_start(t[:], dram[:])
            wts[nm] = t
        iow_t = cst.tile([P, ET_OWN], I32, name="iow_t")
        nc.sync.dma_start(iow_t[:], iow[:])
        jow_t = cst.tile([P, ET_OWN], I32, name="jow_t")
        nc.sync.dma_start(jow_t[:], jow[:])

        acc = ctx.enter_context(tc.tile_pool(name="acc", bufs=1))
        out_acc = acc.tile([1, E], F32, name="out_acc")
        oxa_acc = acc.tile([P, ET_OWN], F32, name="oxa_acc")

        gat = ctx.enter_context(tc.tile_pool(name="gat", bufs=3))
        up_p = ctx.enter_context(tc.tile_pool(name="up_p", bufs=2))
        psp = ctx.enter_context(
            tc.tile_pool(name="psp", bufs=2, space=bass.MemorySpace.PSUM)
        )

        for b in range(reps * NB):
            b = b % NB
            ic = (2 * b) * NIW
            jc = (2 * b + 1) * NIW
            gi = gat.tile([P, C5, E2], BF16, name="gi")
            nc.gpsimd.dma_gather(
                gi[:], bcat[:], idxs_t[:, ic : ic + NIW], E2, E2, BP2,
                transpose=True,
            )
            gj = gat.tile([P, C5, E2], BF16, name="gj")
            nc.gpsimd.dma_gather(
                gj[:], bcat[:], idxs_t[:, jc : jc + NIW], E2, E2, BP2,
                transpose=True,
            )
            t = up_p.tile([P, C5, E2], BF16, name="t")
            nc.vector.tensor_tensor(
                out=t[:].bitcast(U16), in0=gi[:].bitcast(U16),
                in1=gj[:].bitcast(U16), op=AND,
            )
            masks = {}
            for nm, m, sh in (("up", 0x0404, 4), ("uq", 0x0202, 5),
                              ("ur", 0x0101, 6)):
                mt = up_p.tile([P, C5, E2], BF16, name=nm)
                nc.vector.tensor_scalar(
                    out=mt[:].bitcast(U16), in0=t[:].bitcast(U16),
                    scalar1=m, scalar2=sh, op0=AND, op1=SHL,
                )
                masks[nm] = mt
            XOR = mybir.AluOpType.bitwise_xor
            nq = up_p.tile([P, C5, E2], BF16, name="nq")
            nc.vector.tensor_scalar(
                out=nq[:].bitcast(U16), in0=masks["uq"][:].bitcast(U16),
                scalar1=0x4040, scalar2=None, op0=XOR,
            )
            m1 = up_p.tile([P, C5, E2], BF16, name="m1")
            nc.vector.tensor_tensor(
                out=m1[:].bitcast(U16), in0=masks["up"][:].bitcast(U16),
                in1=nq[:].bitcast(U16), op=AND,
            )
            masks["m1"] = m1
            np_ = up_p.tile([P, C5, E2], BF16, name="np")
            nc.vector.tensor_scalar(
                out=np_[:].bitcast(U16), in0=masks["up"][:].bitcast(U16),
                scalar1=0x4040, scalar2=None, op0=XOR,
            )
            m2 = up_p.tile([P, C5, E2], BF16, name="m2")
            nc.vector.tensor_tensor(
                out=m2[:].bitcast(U16), in0=masks["ur"][:].bitcast(U16),
                in1=np_[:].bitcast(U16), op=AND,
            )
            masks["m2"] = m2

            ps = psp.tile([1, E2], F32, name="ps")
            nterm = len(TERMS)
            for ti, term in enumerate(TERMS):
                src = masks[term]
                for c in range(C5):
                    for s in range(2):
                        rhs = src[:].bitcast(FP8)[:, c, s : 2 * E2 : 2]
                        lhsT = wts[term][:, 2 * c + s : 2 * c + s + 1]
                        nc.tensor.matmul(
                            ps[:], lhsT, rhs,
                            start=(ti == 0 and c == 0 and s == 0),
                            stop=(ti == nterm - 1 and c == C5 - 1 and s == 1),
                        )
            nc.scalar.copy(out_acc[:, b * E2 : (b + 1) * E2], ps[:])

        for eo in range(reps * ET_OWN):
            eo = eo % ET_OWN
            xi_t = gat.tile([P, D], F32, name="xi_t")
            nc.gpsimd.indirect_dma_start(
                out=xi_t[:], out_offset=None, in_=xw[:],
                in_offset=bass.IndirectOffsetOnAxis(ap=iow_t[:, eo : eo + 1], axis=0),
            )
            xj_t = gat.tile([P, D], F32, name="xj_t")
            nc.gpsimd.indirect_dma_start(
                out=xj_t[:], out_offset=None, in_=xr[:],
                in_offset=bass.IndirectOffsetOnAxis(ap=jow_t[:, eo : eo + 1], axis=0),
            )
            oxe = up_p.tile([P, D], F32, name="oxe")
            nc.vector.scalar_tensor_tensor(
                out=oxe[:], in0=xi_t[:], scalar=1.0, in1=xj_t[:],
                op0=MUL, op1=MUL, accum_out=oxa_acc[:, eo : eo + 1],
            )

        nc.sync.dma_start(out_cn[:], out_acc[:])
        nc.sync.dma_start(out_xij[:], oxa_acc[:])

    return nc


def get_nc(reps=1):
    key = f"nc{reps}"
    if key not in _CACHE:
        nc = _build_nc(reps)
        nc.compile()
        _CACHE[key] = nc
    return _CACHE[key]


def _wrap16(lst):
    """[P, len//16] int16: idx k at [k % 16 (replicated mod 16), k // 16]."""
    return np.tile(lst.reshape(-1, 16).T, (P // 16, 1))


def make_in_maps(x, adj_0_1, adj_1, adj_0_1_2, tar_ei, Wxs, bxs):
    import ml_dtypes

    bf = ml_dtypes.bfloat16
    f8 = mybir.dt.np(FP8)
    x32 = np.ascontiguousarray(x, dtype=np.float32)
    wxs = np.asarray(Wxs, dtype=np.float32)
    w0 = wxs[0:D, 0]
    wy = np.concatenate(
        [wxs[D : 2 * D], wxs[2 * D : 3 * D], wxs[3 * D : 4 * D]], axis=1
    )
    y = x32 @ wy  # [N, 3]
    xwf = np.ascontiguousarray(x32 * w0[None, :])

    ii16 = tar_ei[0].astype(np.int16)
    jj16 = tar_ei[1].astype(np.int16)
    idxs = np.zeros((P, NB * 2 * NIW), np.int16)
    for b in range(NB):
        es = slice(b * E2, (b + 1) * E2)
        idxs[:, (2 * b) * NIW : (2 * b + 1) * NIW] = _wrap16(ii16[es])
        idxs[:, (2 * b + 1) * NIW : (2 * b + 2) * NIW] = _wrap16(jj16[es])

    a01 = np.asarray(adj_0_1, dtype=np.float32) != 0
    a1 = np.asarray(adj_1, dtype=np.float32) != 0
    a012 = np.asarray(adj_0_1_2, dtype=np.float32) != 0

    ii32 = tar_ei[0].astype(np.int32)
    jj32 = tar_ei[1].astype(np.int32)

    in_maps = []
    for c in range(NCORES):
        c0 = c * NCOL
        cs = slice(c0, c0 + NCOL)
        packed = np.zeros((N, BPAD), dtype=np.uint8)
        packed[:, 0:NCOL] = (
            4 * a01[:, cs] + 2 * a1[:, cs] + a012[:, cs]
        ).astype(np.uint8)
        bcat = packed.view(bf)  # [N, 640]

        ypad = np.zeros((3, BPAD), np.float32)
        ypad[0, 0:NCOL] = y[cs, 0]
        ypad[1, 0:NCOL] = y[cs, 1]
        ypad[2, 0:NCOL] = y[cs, 2]

        # term weights (mask value 2.0 folded in); bf16 weight col 2c+s at
        # partition p weights node g = 256c + 2p + s.
        wsrc = {"m1": 0.5 * ypad[0], "uq": 0.5 * ypad[1], "m2": 0.5 * ypad[2]}
        pidx = np.arange(P)
        maps = {}
        for term, wrow in wsrc.items():
            w = np.zeros((P, 2 * C5), np.float32)
            for ch in range(C5):
                for s in range(2):
                    w[:, 2 * ch + s] = wrow[256 * ch + 2 * pidx + s]
            maps[f"w_{term}"] = np.ascontiguousarray(w.astype(bf))

        esl = slice(c * E_OWN, (c + 1) * E_OWN)
        iow = np.ascontiguousarray(ii32[esl].reshape(ET_OWN, P).T)
        jow = np.ascontiguousarray(jj32[esl].reshape(ET_OWN, P).T)
        in_maps.append({
            "bcat": np.ascontiguousarray(bcat),
            "idxs": idxs,
            **maps,
            "xw": xwf,
            "xr": x32,
            "iow": iow,
            "jow": jow,
        })
    return in_maps


def combine_results(results, b):
    out = np.zeros((E,), dtype=np.float64)
    for c in range(NCORES):
        out += results[c]["out_cn"].astype(np.float64).reshape(E)
        esl = slice(c * E_OWN, (c + 1) * E_OWN)
        out[esl] += results[c]["out_xij"].astype(np.float64).T.reshape(E_OWN)
    return (out + b).astype(np.float32).reshape(E, 1)


def kernel(x, adj_0_1, adj_1, adj_0_1_2, tar_ei, Wxs, bxs):
    nc = get_nc()
    in_maps = make_in_maps(x, adj_0_1, adj_1, adj_0_1_2, tar_ei, Wxs, bxs)
    res = run_bass_kernel_spmd(nc, in_maps, list(range(NCORES)))
    b = float(np.asarray(bxs, dtype=np.float32).reshape(-1)[0])
    return combine_results(res.results, b)


# revision 23
# speedup vs baseline: 27.2446x; 1.3341x over previous
"""NCNPredictor v8: 3-bit-packed adjacency, transposed dma_gather, PE sums.

Per edge (i,j), per core's 1250-column slice, y_k = x @ Wxs[(k+1)D:(k+2)D]:
  out = S p y0 + S q y1 + S r y2 - S p q y0 - S p r y2 + xij + b
  p = A01[i]A01[j], q = A1[i]A1[j], r = A012[i]A012[j]

All three adjacency bits of node n live in ONE byte: v = 4*A01 + 2*A1 +
A012 (zero-padded 1250->1280 cols), exposed as a bf16-typed [N, 640]
tensor (the gather and bitwise ops never interpret values). Per 512-edge
batch, two dma_gather(transpose=True) calls fetch i-rows and j-rows
n-on-partitions: gt[p, c, e] is the u16 byte-pair for nodes
n = 256c + 2p + {0,1}. Then, all exact:
  t  = gi & gj                 (packed p,q,r product bits per byte)
  up = (t & 0x0404) << 4       (p-mask as fp8 bytes {0, 2.0})
  uq = (t & 0x0202) << 5       (q-mask)
  ur = (t & 0x0101) << 6       (r-mask)
  m1 = up & (uq ^ 0x4040)      (p & ~q mask — the reference's cn_0)
  m2 = ur & (up ^ 0x4040)      (r & ~p mask — cn_2)
The PE contracts over n per (chunk, parity) with bf16 weight columns
(w = y_k/2, mask value 2.0 folded in): three plain fp8-rhs matmul passes
(m1*y0, uq*y1, m2*y2) accumulate into one PSUM row per batch; ACT copies
PSUM->SBUF. Bias added host-side.
"""

import sys
from contextlib import ExitStack

import numpy as np

sys.path.insert(0, "/opt/trn_rl_repo")

import concourse.bass as bass
import concourse.tile as tile
from concourse import bacc, library_config, mybir
from concourse.bass_utils import run_bass_kernel_spmd

N = 10000
D = 128
E = 8192
NCORES = 8
NCOL = N // NCORES          # 1250 data cols (nodes) per core
BPAD = 1280                 # padded width (bytes per row)
BP2 = BPAD // 2             # 640 bf16 units per row
C5 = BPAD // 256            # 5 u16 chunks
P = 128
E2 = 512                    # edges per batch
NB = E // E2                # 16 batches
NIW = E2 // 16              # idx cols per gather list (16-wrap)
E_OWN = E // NCORES
ET_OWN = E_OWN // P
F32 = mybir.dt.float32
BF16 = mybir.dt.bfloat16
FP8 = mybir.dt.float8e4
U16 = mybir.dt.uint16
I16 = mybir.dt.int16
I32 = mybir.dt.int32
MUL = mybir.AluOpType.mult
AND = mybir.AluOpType.bitwise_and
SHL = mybir.AluOpType.logical_shift_left

_CACHE = {}

# (mask name, weight set) per PE pass, in PSUM-accumulation order
TERMS = ("m1", "uq", "m2")


def _build_nc(reps=1):
    nc = bacc.Bacc(num_swdge_queues=4)

    bcat = nc.declare_dram_parameter("bcat", [N, BP2], BF16, False)
    idxs = nc.declare_dram_parameter("idxs", [P, NB * 2 * NIW], I16, False)
    wd = {
        t: nc.declare_dram_parameter(f"w_{t}", [P, 2 * C5], BF16, False)
        for t in TERMS
    }
    xw = nc.declare_dram_parameter("xw", [N, D], F32, False)
    xr = nc.declare_dram_parameter("xr", [N, D], F32, False)
    iow = nc.declare_dram_parameter("iow", [P, ET_OWN], I32, False)
    jow = nc.declare_dram_parameter("jow", [P, ET_OWN], I32, False)

    out_cn = nc.declare_dram_parameter("out_cn", [1, E], F32, True)
    out_xij = nc.declare_dram_parameter("out_xij", [P, ET_OWN], F32, True)

    with tile.TileContext(nc) as tc, ExitStack() as ctx:
        nc.gpsimd.load_library(library_config.mlp)

        cst = ctx.enter_context(tc.tile_pool(name="cst", bufs=1))
        idxs_t = cst.tile([P, NB * 2 * NIW], I16, name="idxs_t")
        nc.sync.dma_start(idxs_t[:], idxs[:])
        wts = {}
        for nm, dram in wd.items():
            t = cst.tile([P, 2 * C5], BF16, name=f"wt_{nm}")
            nc.sync.dma_start(t[:], dram[:])
            wts[nm] = t
        iow_t = cst.tile([P, ET_OWN], I32, name="iow_t")
        nc.sync.dma_start(iow_t[:], iow[:])
        jow_t = cst.tile([P, ET_OWN], I32, name="jow_t")
        nc.sync.dma_start(jow_t[:], jow[:])

        acc = ctx.enter_context(tc.tile_pool(name="acc", bufs=1))
        out_acc = acc.tile([1, E], F32, name="out_acc")
        oxa_acc = acc.tile([P, ET_OWN], F32, name="oxa_acc")

        gat = ctx.enter_context(tc.tile_pool(name="gat", bufs=3))
        up_p = ctx.enter_context(tc.tile_pool(name="up_p", bufs=2))
        psp = ctx.enter_context(
            tc.tile_pool(name="psp", bufs=2, space=bass.MemorySpace.PSUM)
        )

        for b in range(reps * NB):
            b = b % NB
            ic = (2 * b) * NIW
            jc = (2 * b + 1) * NIW
            gi = gat.tile([P, C5, E2], BF16, name="gi")
            nc.gpsimd.dma_gather(
                gi[:], bcat[:], idxs_t[:, ic : ic + NIW], E2, E2, BP2,
                transpose=True,
            )
            gj = gat.tile([P, C5, E2], BF16, name="gj")
            nc.gpsimd.dma_gather(
                gj[:], bcat[:], idxs_t[:, jc : jc + NIW], E2, E2, BP2,
                transpose=True,
            )
            t = up_p.tile([P, C5, E2], BF16, name="t")
            nc.vector.tensor_tensor(
                out=t[:].bitcast(U16), in0=gi[:].bitcast(U16),
                in1=gj[:].bitcast(U16), op=AND,
            )
            masks = {}
            for nm, m, sh in (("up", 0x0404, 4), ("uq", 0x0202, 5),
                              ("ur", 0x0101, 6)):
                mt = up_p.tile([P, C5, E2], BF16, name=nm)
                nc.vector.tensor_scalar(
                    out=mt[:].bitcast(U16), in0=t[:].bitcast(U16),
                    scalar1=m, scalar2=sh, op0=AND, op1=SHL,
                )
                masks[nm] = mt
            XOR = mybir.AluOpType.bitwise_xor
            nq = up_p.tile([P, C5, E2], BF16, name="nq")
            nc.vector.tensor_scalar(
                out=nq[:].bitcast(U16), in0=masks["uq"][:].bitcast(U16),
                scalar1=0x4040, scalar2=None, op0=XOR,
            )
            m1 = up_p.tile([P, C5, E2], BF16, name="m1")
            nc.vector.tensor_tensor(
                out=m1[:].bitcast(U16), in0=masks["up"][:].bitcast(U16),
                in1=nq[:].bitcast(U16), op=AND,
            )
            masks["m1"] = m1
            np_ = up_p.tile([P, C5, E2], BF16, name="np")
            nc.vector.tensor_scalar(
                out=np_[:].bitcast(U16), in0=masks["up"][:].bitcast(U16),
                scalar1=0x4040, scalar2=None, op0=XOR,
            )
            m2 = up_p.tile([P, C5, E2], BF16, name="m2")
            nc.vector.tensor_tensor(
                out=m2[:].bitcast(U16), in0=masks["ur"][:].bitcast(U16),
                in1=np_[:].bitcast(U16), op=AND,
            )
            masks["m2"] = m2

            ps = psp.tile([1, E2], F32, name="ps")
            nterm = len(TERMS)
            for ti, term in enumerate(TERMS):
                src = masks[term]
                for c in range(C5):
                    for s in range(2):
                        rhs = src[:].bitcast(FP8)[:, c, s : 2 * E2 : 2]
                        lhsT = wts[term][:, 2 * c + s : 2 * c + s + 1]
                        nc.tensor.matmul(
                            ps[:], lhsT, rhs,
                            start=(ti == 0 and c == 0 and s == 0),
                            stop=(ti == nterm - 1 and c == C5 - 1 and s == 1),
                        )
            nc.scalar.copy(out_acc[:, b * E2 : (b + 1) * E2], ps[:])

        for eo in range(reps * ET_OWN):
            eo = eo % ET_OWN
            xi_t = gat.tile([P, D], F32, name="xi_t")
            nc.gpsimd.indirect_dma_start(
                out=xi_t[:], out_offset=None, in_=xw[:],
                in_offset=bass.IndirectOffsetOnAxis(ap=iow_t[:, eo : eo + 1], axis=0),
            )
            xj_t = gat.tile([P, D], F32, name="xj_t")
            nc.gpsimd.indirect_dma_start(
                out=xj_t[:], out_offset=None, in_=xr[:],
                in_offset=bass.IndirectOffsetOnAxis(ap=jow_t[:, eo : eo + 1], axis=0),
            )
            oxe = up_p.tile([P, D], F32, name="oxe")
            nc.vector.scalar_tensor_tensor(
                out=oxe[:], in0=xi_t[:], scalar=1.0, in1=xj_t[:],
                op0=MUL, op1=MUL, accum_out=oxa_acc[:, eo : eo + 1],
            )

        nc.sync.dma_start(out_cn[:], out_acc[:])
        nc.sync.dma_start(out_xij[:], oxa_acc[:])

    return nc


def get_nc(reps=1):
    key = f"nc{reps}"
    if key not in _CACHE:
        nc = _build_nc(reps)
        nc.compile()
        _CACHE[key] = nc
    return _CACHE[key]


def _wrap16(lst):
    """[P, len//16] int16: idx k at [k % 16 (replicated mod 16), k // 16]."""
    return np.tile(lst.reshape(-1, 16).T, (P // 16, 1))


def make_in_maps(x, adj_0_1, adj_1, adj_0_1_2, tar_ei, Wxs, bxs):
    import ml_dtypes

    bf = ml_dtypes.bfloat16
    f8 = mybir.dt.np(FP8)
    tar_ei = np.asarray(tar_ei)
    x32 = np.ascontiguousarray(x, dtype=np.float32)
    wxs = np.asarray(Wxs, dtype=np.float32)
    w0 = wxs[0:D, 0]
    wy = np.concatenate(
        [wxs[D : 2 * D], wxs[2 * D : 3 * D], wxs[3 * D : 4 * D]], axis=1
    )
    y = x32 @ wy  # [N, 3]
    xwf = np.ascontiguousarray(x32 * w0[None, :])

    ii16 = tar_ei[0].astype(np.int16)
    jj16 = tar_ei[1].astype(np.int16)
    idxs = np.zeros((P, NB * 2 * NIW), np.int16)
    for b in range(NB):
        es = slice(b * E2, (b + 1) * E2)
        idxs[:, (2 * b) * NIW : (2 * b + 1) * NIW] = _wrap16(ii16[es])
        idxs[:, (2 * b + 1) * NIW : (2 * b + 2) * NIW] = _wrap16(jj16[es])

    a01 = np.asarray(adj_0_1, dtype=np.float32) != 0
    a1 = np.asarray(adj_1, dtype=np.float32) != 0
    a012 = np.asarray(adj_0_1_2, dtype=np.float32) != 0

    ii32 = tar_ei[0].astype(np.int32)
    jj32 = tar_ei[1].astype(np.int32)

    in_maps = []
    for c in range(NCORES):
        c0 = c * NCOL
        cs = slice(c0, c0 + NCOL)
        packed = np.zeros((N, BPAD), dtype=np.uint8)
        packed[:, 0:NCOL] = (
            4 * a01[:, cs] + 2 * a1[:, cs] + a012[:, cs]
        ).astype(np.uint8)
        bcat = packed.view(bf)  # [N, 640]

        ypad = np.zeros((3, BPAD), np.float32)
        ypad[0, 0:NCOL] = y[cs, 0]
        ypad[1, 0:NCOL] = y[cs, 1]
        ypad[2, 0:NCOL] = y[cs, 2]

        # term weights (mask value 2.0 folded in); bf16 weight col 2c+s at
        # partition p weights node g = 256c + 2p + s.
        wsrc = {"m1": 0.5 * ypad[0], "uq": 0.5 * ypad[1], "m2": 0.5 * ypad[2]}
        pidx = np.arange(P)
        maps = {}
        for term, wrow in wsrc.items():
            w = np.zeros((P, 2 * C5), np.float32)
            for ch in range(C5):
                for s in range(2):
                    w[:, 2 * ch + s] = wrow[256 * ch + 2 * pidx + s]
            maps[f"w_{term}"] = np.ascontiguousarray(w.astype(bf))

        esl = slice(c * E_OWN, (c + 1) * E_OWN)
        iow = np.ascontiguousarray(ii32[esl].reshape(ET_OWN, P).T)
        jow = np.ascontiguousarray(jj32[esl].reshape(ET_OWN, P).T)
        in_maps.append({
            "bcat": np.ascontiguousarray(bcat),
            "idxs": idxs,
            **maps,
            "xw": xwf,
            "xr": x32,
            "iow": iow,
            "jow": jow,
        })
    return in_maps


def combine_results(results, b):
    out = np.zeros((E,), dtype=np.float64)
    for c in range(NCORES):
        out += results[c]["out_cn"].astype(np.float64).reshape(E)
        esl = slice(c * E_OWN, (c + 1) * E_OWN)
        out[esl] += results[c]["out_xij"].astype(np.float64).T.reshape(E_OWN)
    return (out + b).astype(np.float32).reshape(E, 1)


def kernel(x, adj_0_1, adj_1, adj_0_1_2, tar_ei, Wxs, bxs):
    nc = get_nc()
    in_maps = make_in_maps(x, adj_0_1, adj_1, adj_0_1_2, tar_ei, Wxs, bxs)
    res = run_bass_kernel_spmd(nc, in_maps, list(range(NCORES)))
    b = float(np.asarray(bxs, dtype=np.float32).reshape(-1)[0])
    return combine_results(res.results, b)
